# revision 44
# baseline (speedup 1.0000x reference)
"""DepthDC fused kernel for 8 Trainium2 NeuronCores.

Reference computation (N=2, C=64, H=W=256, d=2):
  patches[n,c,k,h,w] = xpad[n,c,h+ki*d, w+kj*d]   (k=3*ki+kj, pad d)
  out1 = sum_k patches * y.reshape(N,C,9,H,W)
  out  = leaky_relu(conv3x3(out1, fuse_w) + fuse_b, 0.2)

Sharding: 8 cores = batch(2) x H-quarters(4). Each core produces a
[64, 64, 256] output slab. Host slices overlapping (haloed, zero-padded)
input slabs per core, so no device collectives are needed.

Per-core layout: the 64 output rows split into two 32-row halves mapped
to SBUF partition halves (partition = c + 64*s). All on-chip data is
fp16 (PSUM accumulation in fp32); the host casts inputs to fp16 and the
fp16 output back to fp32. y is host-packed as [(s c), q, k, w] so each
per-chunk DMA is one 18KB-contiguous descriptor per partition.

Engines:
  - DVE: 9 elementwise products per chunk (fp16, 2x mode)
  - k-reduction: per-chunk either PE (identity matmul, PSUM acc) or
    DVE (tree of fp16 adds), set by REDUCE_MODE to balance engine load
  - PE:  3x3 dense conv as 9 accumulating matmuls over C=64 (fp16)
  - ACT: PSUM->SBUF out1 copies (PE-mode chunks) and the whole
    bias+leaky-relu epilogue as one Prelu activation per chunk
Work is streamed over 4-row h-chunks with triple-buffered y DMA.
"""

import sys

sys.path.insert(0, "/opt/trn_rl_repo")

from contextlib import ExitStack

import numpy as np

import concourse.bass as bass
import concourse.mybir as mybir
import concourse.tile as tile
from concourse import bacc
from concourse.ap import AP
from concourse.bass_utils import run_bass_kernel_spmd

F16 = mybir.dt.float16
F32 = mybir.dt.float32
AF = mybir.ActivationFunctionType

N, C, H, W = 2, 64, 256, 256
D = 2  # dilation == pad
NEG_SLOPE = 0.2
NCORES = 8
HB = 64          # output rows per core
HH = 32          # output rows per half
Q = HH + 2       # out1 rows per half (34)
XR = Q + 4       # x rows per half block (38)
XW = W + 2 * D   # padded x width (260)
OW = W + 2       # padded out1 width (258)
RC = 4           # rows per chunk
NCONV = 8        # conv chunks per half: 8 x 4 rows

# Reduce chunks: a small first chunk so compute starts as soon as a
# cheap y DMA lands, 4-row chunks in the middle, and two small final
# chunks so little compute remains after the last y DMA lands.
RCS = [2, 4, 4, 4, 4, 4, 4, 4, 2, 2]
Q0S = [0, 2, 6, 10, 14, 18, 22, 26, 30, 32]
NRED = len(RCS)
XHEAD = 14       # x rows in the head tile (covers chunks 0-2)
XTOFF = 10       # first row of the x tail tile (chunks 3+, rows 10-37)
# Which engine reduces the 9 products per chunk: "pe" = identity matmul
# with PSUM accumulation, "dve" = tree of fp16 adds on DVE. Chunk 0 must
# be "dve" (the id/w weights DMA lands after its matmuls would issue);
# the tail chunks are "dve" so the PE can drain the conv backlog while
# the DVE absorbs the last reductions.
REDUCE_MODE = ["dve", "pe", "pe", "pe", "pe", "pe", "pe", "pe", "dve",
               "dve"]
# GpSimd offload was tried and reverted: its adds run 2.4-6us each on HW
# (vs 0.65us on DVE) and the SBUF port pressure slowed DVE multiplies
# from ~645ns to ~845ns. Keep everything off GpSimd.
POOL_PRESUM = False
# Skip the per-matmul LDWEIGHTS when consecutive matmuls share lhsT,
# pairing a standalone ldweights() with non-self-loading matmuls (valid
# for 16-bit dtypes only; fp32/f32r variants are known-broken).
ELIDE_LDW = True


def _build_program():
    nc = bacc.Bacc("TRN2", target_bir_lowering=False, debug=False,
                   num_devices=NCORES)

    xp_d = nc.dram_tensor("xp", [128, XR, XW], F16, kind="ExternalInput").ap()
    yp_d = nc.dram_tensor("yp", [128, Q, 9, W], F16,
                          kind="ExternalInput").ap()
    wt_d = nc.dram_tensor("wt", [128, 9, 128], F16,
                          kind="ExternalInput").ap()
    id_d = nc.dram_tensor("ident", [128, 128], F16, kind="ExternalInput").ap()
    b_d = nc.dram_tensor("bias", [128, 1], F32, kind="ExternalInput").ap()
    out_d = nc.dram_tensor("out", [128, HH, W], F16, kind="ExternalOutput").ap()

    with tile.TileContext(nc) as tc:
        with ExitStack() as ctx:
            const = ctx.enter_context(tc.tile_pool(name="const", bufs=1))
            y_pool = ctx.enter_context(tc.tile_pool(name="y_pool", bufs=4))
            p_pool = ctx.enter_context(tc.tile_pool(name="p_pool", bufs=6))
            a_pool = ctx.enter_context(tc.tile_pool(name="a_pool", bufs=2))
            ps1_pool = ctx.enter_context(
                tc.tile_pool(name="ps1_pool", bufs=2, space="PSUM"))
            ps2_pool = ctx.enter_context(
                tc.tile_pool(name="ps2_pool", bufs=2, space="PSUM"))

            # constants / whole-slab x / whole-slab out1 / whole-slab out.
            # DMA queue is FIFO across all 16 engines, so dispatch order is
            # priority order: x head rows + first y chunks go first so the
            # DVE can start within ~8us; bulk x tail and weights follow.
            w_sb = const.tile([128, 9, 128], F16, name="w_sb")
            id_sb = const.tile([128, 128], F16, name="id_sb")
            b_sb = const.tile([128, 1], F32, name="b_sb")
            # x in two independent tiles (head: chunks 0-2, tail: 3+);
            # rows 10-13 are fetched into both so no chunk's window
            # straddles a tile boundary.
            x_hd = const.tile([128, XHEAD, XW], F16, name="x_hd")
            x_tl = const.tile([128, XR - XTOFF, XW], F16, name="x_tl")
            o1_sb = const.tile([128, Q, OW], F16, name="o1_sb")
            o_all = const.tile([128, HH, W], F16, name="o_all")

            # zero the conv W-padding columns once. memzero bitcasts to
            # uint32 so SBUF residue can't turn 0*Inf into NaN; cols 1 and
            # 256 are also cleared but every chunk overwrites them later.
            nc.scalar.memzero(o1_sb[:, :, 0:2])
            nc.scalar.memzero(o1_sb[:, :, OW - 2:OW])
            # Wait-merge scratch: one cheap DVE copy per input DMA converts
            # DMA-completion semaphores into DVE program order, so compute
            # instructions never need more than 1 foreign wait sem (the
            # TT-struct wait-slot limit in walrus codegen is tight). Each
            # copy is placed just before the first consumer chunk so the
            # DVE never blocks on a DMA earlier than necessary.
            scr = const.tile([128, 8], F16, name="scr")

            y_tiles = {}

            def issue_y(c):
                if c >= NRED:
                    return
                q0, rc = Q0S[c], RCS[c]
                y_t = y_pool.tile([128, RC, 9, W], F16, name="y_t", tag="y_t")
                nc.sync.dma_start(y_t[:, 0:rc], yp_d[:, q0:q0 + rc])
                y_tiles[c] = y_t

            issue_y(0)
            nc.sync.dma_start(x_hd[:], xp_d[:, 0:XHEAD, :])
            nc.sync.dma_start(id_sb[:], id_d)
            nc.sync.dma_start(b_sb[:], b_d)
            issue_y(1)
            nc.sync.dma_start(w_sb[:], wt_d)
            nc.sync.dma_start(x_tl[:], xp_d[:, XTOFF:XR, :])
            issue_y(2)
            nc.vector.tensor_copy(scr[:, 0:1], x_hd[:, 0, 0:1])

            def x3_view(ki, q0, rc):
                # overlapping window [128, 3(kj, stride 2), rc, W]: the
                # three same-row taps of one product batch share one AP
                r = q0 + 2 * ki
                xt = x_hd if q0 + 7 < XHEAD else x_tl
                if xt is x_tl:
                    r -= XTOFF
                base = xt[:, r:r + rc, 0:W]
                return AP(base.tensor, base.offset,
                          [[base.ap[0][0], 128], [2, 3], [XW, rc], [1, W]])

            def y3_view(y_t, ki, rc):
                # y_t is [128, RC, 9, W]; pick taps 3*ki..3*ki+2 with the
                # tap axis leading: [128, 3, rc, W]
                return y_t[:, 0:rc, 3 * ki:3 * ki + 3, :].transpose(
                    [0, 2, 1, 3])

            def mult3(ki, q0, rc, y_t):
                # one DVE op for the three same-row products
                p3 = p_pool.tile([128, 3, RC, W], F16, name="p3", tag="p3")
                nc.vector.tensor_mul(p3[:, :, 0:rc, :], x3_view(ki, q0, rc),
                                     y3_view(y_t, ki, rc))
                return p3

            def reduce_chunk(c):
                q0, rc = Q0S[c], RCS[c]
                issue_y(c + 3)
                y_t = y_tiles.pop(c)
                nc.vector.tensor_copy(scr[:, 5:6], y_t[:, 0, 0, 0:1])
                if c == 1:
                    nc.vector.tensor_copy(scr[:, 3:4], id_sb[:, 0:1])
                    nc.vector.tensor_copy(
                        scr[:, 4:5], b_sb[:, 0:1].bitcast(F16)[:, 0:1])
                elif c == 2:
                    nc.vector.tensor_copy(scr[:, 2:3], w_sb[:, 0, 0:1])
                elif c == 3:
                    nc.vector.tensor_copy(scr[:, 1:2],
                                          x_tl[:, XR - XTOFF - 1, 0:1])
                if REDUCE_MODE[c] == "pe":
                    ps1 = ps1_pool.tile([128, RC, W], F32, name="ps1",
                                        tag="ps1")
                    # all products first, then a dense matmul stream
                    p3s = [mult3(ki, q0, rc, y_t) for ki in range(3)]
                    if ELIDE_LDW:
                        nc.tensor.ldweights(id_sb[:])
                    for k in range(9):
                        ki, kj = divmod(k, 3)
                        for g in range(rc // 2):
                            r0 = 2 * g
                            mm = nc.tensor.matmul(
                                ps1[:, r0:r0 + 2, :], lhsT=id_sb[:],
                                rhs=p3s[ki][:, kj, r0:r0 + 2, :],
                                start=(k == 0), stop=(k == 8))
                            if ELIDE_LDW:
                                mm.ldweights = False
                    nc.scalar.copy(o1_sb[:, q0:q0 + rc, 1:W + 1],
                                   ps1[:, 0:rc, :])
                else:
                    # 3-wide product batches + adds, all on DVE:
                    # a3 = p3(0)+p3(1)+p3(2), then fold the 3 taps
                    a3 = a_pool.tile([128, 3, RC, W], F16, name="a3",
                                     tag="a3")
                    p3_0 = mult3(0, q0, rc, y_t)
                    p3_1 = mult3(1, q0, rc, y_t)
                    nc.vector.tensor_add(a3[:, :, 0:rc, :],
                                         p3_0[:, :, 0:rc, :],
                                         p3_1[:, :, 0:rc, :])
                    p3_2 = mult3(2, q0, rc, y_t)
                    nc.vector.tensor_add(a3[:, :, 0:rc, :],
                                         a3[:, :, 0:rc, :],
                                         p3_2[:, :, 0:rc, :])
                    acc = a_pool.tile([128, RC, W], F16, name="acc",
                                      tag="acc")
                    nc.vector.tensor_add(acc[:, 0:rc, :],
                                         a3[:, 0, 0:rc, :],
                                         a3[:, 1, 0:rc, :])
                    nc.vector.tensor_add(o1_sb[:, q0:q0 + rc, 1:W + 1],
                                         acc[:, 0:rc, :],
                                         a3[:, 2, 0:rc, :])

            def conv_chunk(j):
                m0 = RC * j
                ps2 = ps2_pool.tile([128, RC, W], F32, name="ps2", tag="ps2")
                # the last chunk runs its two 2-row groups back to back
                # with per-group epilogue+DMA so the final tail is short
                groups = ((0, 1),) if j < NCONV - 1 else ((0,), (1,))
                for gs in groups:
                    for t in range(9):
                        i3, j3 = divmod(t, 3)
                        if ELIDE_LDW:
                            nc.tensor.ldweights(w_sb[:, t])
                        for g in gs:
                            r0 = 2 * g
                            mm = nc.tensor.matmul(
                                ps2[:, r0:r0 + 2, :], lhsT=w_sb[:, t],
                                rhs=o1_sb[:, m0 + i3 + r0: m0 + i3 + r0 + 2,
                                          j3: j3 + W],
                                start=(t == 0), stop=(t == 8))
                            if ELIDE_LDW:
                                mm.ldweights = False
                    if j == NCONV - 1:
                        # epilogue on ACT: prelu(v + b, 0.2), v = conv psum
                        r0, r1 = 2 * gs[0], 2 * gs[-1] + 2
                        nc.scalar.activation(
                            o_all[:, m0 + r0:m0 + r1, :],
                            ps2[:, r0:r1, :], AF.Prelu, bias=b_sb[:, 0:1],
                            scale=1.0, alpha=NEG_SLOPE)
                        nc.sync.dma_start(out_d[:, m0 + r0:m0 + r1, :],
                                          o_all[:, m0 + r0:m0 + r1, :])
                if j < NCONV - 1:
                    nc.scalar.activation(o_all[:, m0:m0 + RC, :], ps2[:],
                                         AF.Prelu, bias=b_sb[:, 0:1],
                                         scale=1.0, alpha=NEG_SLOPE)
                    # 8-row output DMAs; penultimate chunk flushes its own
                    # 4 rows so only 2+2 rows remain after the last chunk
                    if j in (1, 3, 5):
                        g0 = m0 - RC
                        nc.sync.dma_start(out_d[:, g0:g0 + 2 * RC, :],
                                          o_all[:, g0:g0 + 2 * RC, :])
                    elif j == 6:
                        nc.sync.dma_start(out_d[:, m0:m0 + RC, :],
                                          o_all[:, m0:m0 + RC, :])

            # conv trails the reduce by 2 chunks so its o1 rows (and the
            # ACT PSUM->SBUF copy that produces them) are long since done
            # when the PE reaches the conv matmuls — no mid-stream stalls.
            for c in range(NRED):
                reduce_chunk(c)
                if 2 <= c < NCONV + 2:
                    conv_chunk(c - 2)

    nc.compile()
    return nc


_PROGRAM = None


def _get_program():
    global _PROGRAM
    if _PROGRAM is None:
        _PROGRAM = _build_program()
    return _PROGRAM


def make_in_maps(x, y, fuse_w, fuse_b):
    x = np.asarray(x, dtype=np.float32)
    y = np.asarray(y, dtype=np.float32)
    fuse_w = np.asarray(fuse_w, dtype=np.float32)
    fuse_b = np.asarray(fuse_b, dtype=np.float32)

    # x padded to fp16 [N, C, H+6, W+4]: row offset +3, col offset +2
    xf = np.zeros((N, C, H + 6, W + 4), np.float16)
    xf[:, :, 3:3 + H, 2:2 + W] = x
    # y as fp16 [N, C, 9, H+2, W]: row offset +1
    yf = np.zeros((N, C, 9, H + 2, W), np.float16)
    yf[:, :, :, 1:1 + H, :] = y.reshape(N, C, 9, H, W)

    # block-diagonal conv weights: each partition half (h-half of the
    # slab) contracts with its own copy of W_tap in one K=128 matmul.
    # Stored [p, t, m] so the device DMA is contiguous per partition.
    wt = np.zeros((9, 128, 128), np.float16)
    for t in range(9):
        i, j = divmod(t, 3)
        wtap = fuse_w[:, :, i, j].T  # [c_in, c_out]
        wt[t, 0:64, 0:64] = wtap
        wt[t, 64:128, 64:128] = wtap
    wt = np.ascontiguousarray(wt.transpose(1, 0, 2))
    ident = np.eye(128, dtype=np.float16)
    bias = np.concatenate([fuse_b, fuse_b]).astype(np.float32)[:, None]

    in_maps = []
    for core in range(NCORES):
        n, hb = divmod(core, 4)
        h0 = hb * HB
        # x slab [(s c), 38, 260]
        xp = np.concatenate(
            [xf[n, :, h0:h0 + XR, :], xf[n, :, h0 + HH:h0 + HH + XR, :]],
            axis=0)
        xp = np.ascontiguousarray(xp)
        # y slab [(s c), 34, 9, 256]: rows h0+32s-1+q (offset +1 in yf)
        yhalves = []
        for s in (0, 1):
            r0 = h0 + HH * s
            yh = yf[n, :, :, r0:r0 + Q, :]          # [C, 9, Q, W]
            yhalves.append(yh.transpose(0, 2, 1, 3))  # [C, Q, 9, W]
        yp = np.ascontiguousarray(np.concatenate(yhalves, axis=0))
        in_maps.append({"xp": xp, "yp": yp, "wt": wt, "ident": ident,
                        "bias": bias})
    return in_maps


def run(x, y, fuse_w, fuse_b, trace=False, **kw):
    nc = _get_program()
    in_maps = make_in_maps(x, y, fuse_w, fuse_b)
    res = run_bass_kernel_spmd(nc, in_maps, list(range(NCORES)),
                               trace=trace, **kw)
    out = np.empty((N, C, H, W), np.float32)
    for core in range(NCORES):
        n, hb = divmod(core, 4)
        h0 = hb * HB
        r = np.asarray(res.results[core]["out"], dtype=np.float32)
        out[n, :, h0:h0 + HH, :] = r[0:64]
        out[n, :, h0 + HH:h0 + HB, :] = r[64:128]
    return out, res


def kernel(x, y, fuse_w, fuse_b):
    out, _ = run(x, y, fuse_w, fuse_b, trace=False)
    return out
